# revision 8
# baseline (speedup 1.0000x reference)
"""Batched voxel-grid subsampling kernel for 8 Trainium2 NeuronCores.

Contract: kernel(**inputs) takes the FULL inputs (batch_points [N,3] f32,
batch_feats [N,64] f32, batch_len [17] i32) and returns the FULL outputs
(pool_points [N,3] f32, pool_feats [N,64] f32, pool_batch [17] i32),
matching reference.reference() bit-closely.

Semantics note: the reference runs under default JAX config (x64 disabled),
so its int64 voxel-hash key silently truncates to int32 and wraps:
  key = ((bid*2^18 + vx)*2^18 + vy)*2^18 + vz  (mod 2^32)  ==  vy*2^18 + vz
i.e. the grouping collapses to (vy, vz) bins shared across all clouds
(625 bins for unit-cube inputs). The host replicates the reference's index
pipeline verbatim with jnp-on-CPU (so any env-dependent overflow/shift
behavior matches by construction), then the heavy work — the segmented
sum of 1M x 67 f32 rows into the bins — runs on the 8 NeuronCores:

  - Host sorts rows by bin and packs them transposed:
      feats: partition = feature + 64*half,  free = (bin, position)
      points: partition = coord*32 + fold,   free = (bin, position)
    with every bin zero-padded to a uniform length L (exact: +0.0 adds).
  - Each core DMA-streams its ~39MB shard and does exact-f32
    vector.tensor_reduce row sums plus a reciprocal-count scale.
  - Host adds the 2 half-partials (feats) / 32 fold-partials (points)
    and scatters the rows into the padded output arrays.
"""

import os
import numpy as np

DL = 0.04
SHIFT = 18
M = 1 << SHIFT
NCORES = 8

last_exec_time_ns = None  # set when BGS_TRACE=1

_compile_cache = {}


def _cpu_device():
    import jax

    return jax.devices("cpu")[0]


def _index_pipeline(batch_points, batch_len):
    """Replicates the reference's key computation verbatim (jnp on CPU)."""
    import jax
    import jax.numpy as jnp

    with jax.default_device(_cpu_device()):
        batch_points = jnp.asarray(batch_points)
        batch_len = jnp.asarray(batch_len)
        N = batch_points.shape[0]
        B = batch_len.shape[0] - 1
        pt_idx = jnp.arange(N)
        batch_id = jnp.searchsorted(batch_len[1:], pt_idx, side="right")
        vox = jnp.floor(batch_points / DL).astype(jnp.int64)
        vmin = jax.ops.segment_min(vox, batch_id, num_segments=B)
        v = vox - vmin[batch_id]
        bid = batch_id.astype(jnp.int64)
        key = ((bid * M + v[:, 0]) * M + v[:, 1]) * M + v[:, 2]
        return np.asarray(key)


def _pool_batch(key_u_padded, B):
    """Replicates the reference's pool_batch tail verbatim (jnp on CPU)."""
    import jax
    import jax.numpy as jnp

    with jax.default_device(_cpu_device()):
        key_u = jnp.asarray(key_u_padded)
        valid = key_u >= 0
        ub = jnp.where(valid, key_u >> (3 * SHIFT), B)
        per_batch = jax.ops.segment_sum(
            valid.astype(jnp.int32), ub, num_segments=B + 1
        )[:B]
        pool_batch = jnp.concatenate(
            [jnp.zeros((1,), jnp.int32), jnp.cumsum(per_batch).astype(jnp.int32)]
        )
        return np.asarray(pool_batch)


def _install_ntff_hook():
    """The image's antenv lacks axon_hooks; register the ctypes NTFF hook."""
    import sys
    import types

    if "antenv.axon_hooks" in sys.modules:
        return
    import antenv

    mod = types.ModuleType("antenv.axon_hooks")
    holder = [None]
    mod.set_axon_ntff_profile_hook = lambda h: holder.__setitem__(0, h)
    mod.get_axon_ntff_profile_hook = lambda: holder[0]
    sys.modules["antenv.axon_hooks"] = mod
    antenv.axon_hooks = mod
    try:
        from trn_agent_boot.trn_boot import _ntff_profile_via_ctypes

        mod.set_axon_ntff_profile_hook(
            _ntff_profile_via_ctypes("/opt/axon/libaxon_pjrt.so")
        )
    except Exception:
        pass


def _build_device_program(NB, L, CB):
    """Per-core SPMD program: chunked f32 row-sum reduction + scale."""
    import concourse.bacc as bacc
    import concourse.mybir as mybir
    import concourse.tile as tile

    key = (NB, L, CB)
    if key in _compile_cache:
        return _compile_cache[key]

    LH = L // 2
    LP = L // 32
    NCH = NB // CB
    f32 = mybir.dt.float32

    nc = bacc.Bacc("TRN2", target_bir_lowering=False)
    f2 = nc.dram_tensor("f2", [NCH, 128, CB * LH], f32, kind="ExternalInput")
    p32 = nc.dram_tensor("p32", [96, NB * LP], f32, kind="ExternalInput")
    invf = nc.dram_tensor("invf", [128, NB], f32, kind="ExternalInput")
    invp = nc.dram_tensor("invp", [96, NB], f32, kind="ExternalInput")
    of = nc.dram_tensor("of", [128, NB], f32, kind="ExternalOutput")
    op = nc.dram_tensor("op", [96, NB], f32, kind="ExternalOutput")

    with tile.TileContext(nc) as tc:
        with (
            tc.tile_pool(name="fin", bufs=3) as fin,
            tc.tile_pool(name="misc", bufs=1) as misc,
        ):
            accf = misc.tile([128, NB], f32)
            for ch in range(NCH):
                t = fin.tile([128, CB * LH], f32, tag="fchunk")
                nc.sync.dma_start(t[:], f2[ch])
                nc.vector.tensor_reduce(
                    out=accf[:, ch * CB : (ch + 1) * CB],
                    in_=t[:].rearrange("p (b j) -> p b j", b=CB),
                    axis=mybir.AxisListType.X,
                    op=mybir.AluOpType.add,
                )
            accp = misc.tile([96, NB], f32)
            for ch in range(NCH):
                tp = fin.tile([96, CB * LP], f32, tag="pchunk")
                nc.sync.dma_start(tp[:], p32[:, ch * CB * LP : (ch + 1) * CB * LP])
                nc.vector.tensor_reduce(
                    out=accp[:, ch * CB : (ch + 1) * CB],
                    in_=tp[:].rearrange("p (b j) -> p b j", b=CB),
                    axis=mybir.AxisListType.X,
                    op=mybir.AluOpType.add,
                )
            invf_t = misc.tile([128, NB], f32)
            nc.sync.dma_start(invf_t[:], invf[:])
            invp_t = misc.tile([96, NB], f32)
            nc.sync.dma_start(invp_t[:], invp[:])
            outf = misc.tile([128, NB], f32)
            nc.vector.tensor_tensor(
                out=outf[:], in0=accf[:], in1=invf_t[:], op=mybir.AluOpType.mult
            )
            outp = misc.tile([96, NB], f32)
            nc.vector.tensor_tensor(
                out=outp[:], in0=accp[:], in1=invp_t[:], op=mybir.AluOpType.mult
            )
            nc.sync.dma_start(of[:], outf[:])
            nc.sync.dma_start(op[:], outp[:])

    nc.compile()
    _compile_cache[key] = nc
    return nc


def kernel(batch_points, batch_feats, batch_len):
    global last_exec_time_ns
    from concourse import bass_utils

    batch_points = np.ascontiguousarray(batch_points, dtype=np.float32)
    batch_feats = np.ascontiguousarray(batch_feats, dtype=np.float32)
    batch_len = np.ascontiguousarray(batch_len, dtype=np.int32)
    N = batch_points.shape[0]
    B = batch_len.shape[0] - 1

    # --- host index pipeline (reference-exact) ---
    key = _index_pipeline(batch_points, batch_len)
    key_u, inv = np.unique(key, return_inverse=True)
    inv = inv.astype(np.int64)
    U = key_u.shape[0]
    counts = np.bincount(inv, minlength=U)
    denom = np.maximum(counts, 1).astype(np.float32)
    invc = (np.float32(1.0) / denom).astype(np.float32)

    # Pick a uniform subbin length LS (multiple of 32): every bin is split
    # into ceil(n/LS) subbins packed to exactly LS rows (zero-padded).
    # Minimize total padded rows; cap so a chunk of >=1 subbin fits SBUF.
    maxn = int(counts.max())
    cands = [32 * k for k in range(1, 701) if 32 * k <= max(32, maxn) + 31]
    best = None
    for ls in cands:
        tot = int((-(-counts // ls)).sum()) * ls
        if best is None or tot < best[0]:
            best = (tot, ls)
    LS = best[1]
    LSH = LS // 2
    LSP = LS // 32

    nsb = (-(-counts // LS)).astype(np.int64)  # subbins per bin
    sb_of_bin = np.zeros(U + 1, np.int64)
    np.cumsum(nsb, out=sb_of_bin[1:])
    TSB = int(sb_of_bin[-1])  # total subbins
    sb2bin = np.repeat(np.arange(U, dtype=np.int64), nsb)

    # chunking: CB subbins per DMA chunk, sized for SBUF (<=~44KB/partition)
    CB = max(1, min(16, 11264 // LSH))
    NB = (-(-(-(-TSB // NCORES)) // CB)) * CB  # ceil(ceil(TSB/8)/CB)*CB
    NB = max(NB, CB)
    NCH = NB // CB

    order = np.argsort(inv, kind="stable")
    inv_s = inv[order]
    bin_start = np.zeros(U + 1, np.int64)
    np.cumsum(counts, out=bin_start[1:])
    rank = np.arange(N, dtype=np.int64) - bin_start[inv_s]

    sb = sb_of_bin[inv_s] + rank // LS  # global subbin index per point
    r = rank % LS  # rank within subbin
    core = sb // NB
    lb = sb % NB

    # feats: fixed half-split within each subbin
    h = r // LSH
    j = r - h * LSH
    ch = lb // CB
    bb = lb % CB
    colF = bb * LSH + j

    F2 = np.zeros((NCORES, NCH, 2, 64, CB * LSH), np.float32)
    F2[core, ch, h, :, colF] = batch_feats[order]

    # points: fixed 32-fold split within each subbin
    f = r // LSP
    jp = r - f * LSP
    colP = lb * LSP + jp
    P32 = np.zeros((NCORES, 3, 32, NB * LSP), np.float32)
    P32[core, :, f, colP] = batch_points[order]

    # per-subbin inverse-count tables (replicated across partitions)
    invc_sb = np.zeros(NCORES * NB, np.float32)
    invc_sb[:TSB] = invc[sb2bin]
    invc_sb = invc_sb.reshape(NCORES, 1, NB)
    INVF = np.broadcast_to(invc_sb, (NCORES, 128, NB)).copy()
    INVP = np.broadcast_to(invc_sb, (NCORES, 96, NB)).copy()

    # --- device execution ---
    nc = _build_device_program(NB, LS, CB)
    in_maps = [
        {
            "f2": np.ascontiguousarray(F2[c].reshape(NCH, 128, CB * LSH)),
            "p32": np.ascontiguousarray(P32[c].reshape(96, NB * LSP)),
            "invf": np.ascontiguousarray(INVF[c]),
            "invp": np.ascontiguousarray(INVP[c]),
        }
        for c in range(NCORES)
    ]
    trace = os.environ.get("BGS_TRACE", "0") == "1"
    if trace:
        _install_ntff_hook()
    res = bass_utils.run_bass_kernel_spmd(
        nc, in_maps, core_ids=list(range(NCORES)), trace=trace
    )
    if trace:
        last_exec_time_ns = res.exec_time_ns

    # --- host finalize: combine halves/folds, then subbin partials -> bins ---
    OF = np.stack([res.results[c]["of"] for c in range(NCORES)])  # [8,128,NB]
    OP = np.stack([res.results[c]["op"] for c in range(NCORES)])  # [8,96,NB]
    feats_parts = OF.reshape(NCORES, 2, 64, NB)
    feats_sb = (
        feats_parts[:, 0].transpose(0, 2, 1) + feats_parts[:, 1].transpose(0, 2, 1)
    ).reshape(NCORES * NB, 64)  # per-subbin scaled sums
    pts_parts = OP.reshape(NCORES, 3, 32, NB)
    pts_sb = pts_parts.sum(axis=2).transpose(0, 2, 1).reshape(NCORES * NB, 3)

    pool_points = np.zeros((N, 3), np.float32)
    pool_feats = np.zeros((N, 64), np.float32)
    np.add.at(pool_feats, sb2bin, feats_sb[:TSB])
    np.add.at(pool_points, sb2bin, pts_sb[:TSB])

    key_u_padded = np.full(N, -1, dtype=key_u.dtype)
    key_u_padded[:U] = key_u
    pool_batch = _pool_batch(key_u_padded, B)

    return pool_points, pool_feats, pool_batch


# revision 9
# speedup vs baseline: 3.1736x; 3.1736x over previous
"""Batched voxel-grid subsampling kernel for 8 Trainium2 NeuronCores.

Contract: kernel(**inputs) takes the FULL inputs (batch_points [N,3] f32,
batch_feats [N,64] f32, batch_len [17] i32) and returns the FULL outputs
(pool_points [N,3] f32, pool_feats [N,64] f32, pool_batch [17] i32),
matching reference.reference() bit-closely.

Semantics note: the reference runs under default JAX config (x64 disabled),
so its int64 voxel-hash key silently truncates to int32 and wraps:
  key = ((bid*2^18 + vx)*2^18 + vy)*2^18 + vz  (mod 2^32)  ==  vy*2^18 + vz
i.e. the grouping collapses to (vy, vz) bins shared across all clouds
(625 bins for unit-cube inputs). The host replicates the reference's index
pipeline verbatim with jnp-on-CPU (so any env-dependent overflow/shift
behavior matches by construction), then the heavy work — the segmented
sum of 1M x 67 f32 rows into the bins — runs on the 8 NeuronCores:

  - Host sorts rows by bin and packs them transposed:
      feats: partition = feature + 64*half,  free = (bin, position)
      points: partition = coord*32 + fold,   free = (bin, position)
    with every bin zero-padded to a uniform length L (exact: +0.0 adds).
  - Each core DMA-streams its ~39MB shard and does exact-f32
    vector.tensor_reduce row sums plus a reciprocal-count scale.
  - Host adds the 2 half-partials (feats) / 32 fold-partials (points)
    and scatters the rows into the padded output arrays.
"""

import os
import numpy as np

DL = 0.04
SHIFT = 18
M = 1 << SHIFT
NCORES = 8

last_exec_time_ns = None  # set when BGS_TRACE=1

_compile_cache = {}


def _cpu_device():
    import jax

    return jax.devices("cpu")[0]


def _index_pipeline(batch_points, batch_len):
    """Replicates the reference's key computation verbatim (jnp on CPU)."""
    import jax
    import jax.numpy as jnp

    with jax.default_device(_cpu_device()):
        batch_points = jnp.asarray(batch_points)
        batch_len = jnp.asarray(batch_len)
        N = batch_points.shape[0]
        B = batch_len.shape[0] - 1
        pt_idx = jnp.arange(N)
        batch_id = jnp.searchsorted(batch_len[1:], pt_idx, side="right")
        vox = jnp.floor(batch_points / DL).astype(jnp.int64)
        vmin = jax.ops.segment_min(vox, batch_id, num_segments=B)
        v = vox - vmin[batch_id]
        bid = batch_id.astype(jnp.int64)
        key = ((bid * M + v[:, 0]) * M + v[:, 1]) * M + v[:, 2]
        return np.asarray(key)


def _pool_batch(key_u_padded, B):
    """Replicates the reference's pool_batch tail verbatim (jnp on CPU)."""
    import jax
    import jax.numpy as jnp

    with jax.default_device(_cpu_device()):
        key_u = jnp.asarray(key_u_padded)
        valid = key_u >= 0
        ub = jnp.where(valid, key_u >> (3 * SHIFT), B)
        per_batch = jax.ops.segment_sum(
            valid.astype(jnp.int32), ub, num_segments=B + 1
        )[:B]
        pool_batch = jnp.concatenate(
            [jnp.zeros((1,), jnp.int32), jnp.cumsum(per_batch).astype(jnp.int32)]
        )
        return np.asarray(pool_batch)


def _install_ntff_hook():
    """The image's antenv lacks axon_hooks; register the ctypes NTFF hook."""
    import sys
    import types

    if "antenv.axon_hooks" in sys.modules:
        return
    import antenv

    mod = types.ModuleType("antenv.axon_hooks")
    holder = [None]
    mod.set_axon_ntff_profile_hook = lambda h: holder.__setitem__(0, h)
    mod.get_axon_ntff_profile_hook = lambda: holder[0]
    sys.modules["antenv.axon_hooks"] = mod
    antenv.axon_hooks = mod
    try:
        from trn_agent_boot.trn_boot import _ntff_profile_via_ctypes

        mod.set_axon_ntff_profile_hook(
            _ntff_profile_via_ctypes("/opt/axon/libaxon_pjrt.so")
        )
    except Exception:
        pass


def _build_device_program(NB, L, CB):
    """Per-core SPMD program: chunked f32 row-sum reduction + scale."""
    import concourse.bacc as bacc
    import concourse.mybir as mybir
    import concourse.tile as tile

    key = (NB, L, CB)
    if key in _compile_cache:
        return _compile_cache[key]

    LH = L // 2
    LP = L // 32
    NCH = NB // CB
    f32 = mybir.dt.float32

    nc = bacc.Bacc("TRN2", target_bir_lowering=False)
    f2 = nc.dram_tensor("f2", [NCH, 128, CB * LH], f32, kind="ExternalInput")
    p32 = nc.dram_tensor("p32", [96, NB * LP], f32, kind="ExternalInput")
    invf = nc.dram_tensor("invf", [128, NB], f32, kind="ExternalInput")
    invp = nc.dram_tensor("invp", [96, NB], f32, kind="ExternalInput")
    of = nc.dram_tensor("of", [128, NB], f32, kind="ExternalOutput")
    op = nc.dram_tensor("op", [96, NB], f32, kind="ExternalOutput")

    with tile.TileContext(nc) as tc:
        with (
            tc.tile_pool(name="fin", bufs=3) as fin,
            tc.tile_pool(name="misc", bufs=1) as misc,
        ):
            accf = misc.tile([128, NB], f32)
            for ch in range(NCH):
                t = fin.tile([128, CB * LH], f32, tag="fchunk")
                nc.sync.dma_start(t[:], f2[ch])
                nc.vector.tensor_reduce(
                    out=accf[:, ch * CB : (ch + 1) * CB],
                    in_=t[:].rearrange("p (b j) -> p b j", b=CB),
                    axis=mybir.AxisListType.X,
                    op=mybir.AluOpType.add,
                )
            accp = misc.tile([96, NB], f32)
            for ch in range(NCH):
                tp = fin.tile([96, CB * LP], f32, tag="pchunk")
                nc.sync.dma_start(tp[:], p32[:, ch * CB * LP : (ch + 1) * CB * LP])
                nc.vector.tensor_reduce(
                    out=accp[:, ch * CB : (ch + 1) * CB],
                    in_=tp[:].rearrange("p (b j) -> p b j", b=CB),
                    axis=mybir.AxisListType.X,
                    op=mybir.AluOpType.add,
                )
            invf_t = misc.tile([128, NB], f32)
            nc.sync.dma_start(invf_t[:], invf[:])
            invp_t = misc.tile([96, NB], f32)
            nc.sync.dma_start(invp_t[:], invp[:])
            outf = misc.tile([128, NB], f32)
            nc.vector.tensor_tensor(
                out=outf[:], in0=accf[:], in1=invf_t[:], op=mybir.AluOpType.mult
            )
            outp = misc.tile([96, NB], f32)
            nc.vector.tensor_tensor(
                out=outp[:], in0=accp[:], in1=invp_t[:], op=mybir.AluOpType.mult
            )
            nc.sync.dma_start(of[:], outf[:])
            nc.sync.dma_start(op[:], outp[:])

    nc.compile()
    _compile_cache[key] = nc
    return nc


def kernel(batch_points, batch_feats, batch_len):
    global last_exec_time_ns
    from concourse import bass_utils

    batch_points = np.ascontiguousarray(batch_points, dtype=np.float32)
    batch_feats = np.ascontiguousarray(batch_feats, dtype=np.float32)
    batch_len = np.ascontiguousarray(batch_len, dtype=np.int32)
    N = batch_points.shape[0]
    B = batch_len.shape[0] - 1

    # --- host index pipeline (reference-exact) ---
    key = _index_pipeline(batch_points, batch_len)
    key_u, inv = np.unique(key, return_inverse=True)
    inv = inv.astype(np.int64)
    U = key_u.shape[0]
    counts = np.bincount(inv, minlength=U)
    denom = np.maximum(counts, 1).astype(np.float32)
    invc = (np.float32(1.0) / denom).astype(np.float32)

    # Pick a uniform subbin length LS (multiple of 32): every bin is split
    # into ceil(n/LS) subbins packed to exactly LS rows (zero-padded).
    # Cost = padded rows + per-chunk DMA overhead + per-subbin op overhead,
    # all in padded-row equivalents; cap so a chunk of >=1 subbin fits SBUF.
    maxn = int(counts.max())
    cands = [32 * k for k in range(1, 701) if 32 * k <= max(32, maxn) + 31]
    best = None
    for ls in cands:
        tsb = int((-(-counts // ls)).sum())
        padded = tsb * ls
        cb = max(1, min(16, 11264 // (ls // 2)))
        nch = -(-(-(-tsb // NCORES)) // cb)
        cost = padded + nch * 6000 + tsb * 8
        if best is None or cost < best[0]:
            best = (cost, ls)
    LS = best[1]
    LSH = LS // 2
    LSP = LS // 32

    nsb = (-(-counts // LS)).astype(np.int64)  # subbins per bin
    sb_of_bin = np.zeros(U + 1, np.int64)
    np.cumsum(nsb, out=sb_of_bin[1:])
    TSB = int(sb_of_bin[-1])  # total subbins
    sb2bin = np.repeat(np.arange(U, dtype=np.int64), nsb)

    # chunking: CB subbins per DMA chunk, sized for SBUF (<=~44KB/partition)
    CB = max(1, min(16, 11264 // LSH))
    NB = (-(-(-(-TSB // NCORES)) // CB)) * CB  # ceil(ceil(TSB/8)/CB)*CB
    NB = max(NB, CB)
    NCH = NB // CB

    order = np.argsort(inv, kind="stable")
    inv_s = inv[order]
    bin_start = np.zeros(U + 1, np.int64)
    np.cumsum(counts, out=bin_start[1:])
    rank = np.arange(N, dtype=np.int64) - bin_start[inv_s]

    sb = sb_of_bin[inv_s] + rank // LS  # global subbin index per point
    r = rank % LS  # rank within subbin
    core = sb // NB
    lb = sb % NB

    # feats: fixed half-split within each subbin
    h = r // LSH
    j = r - h * LSH
    ch = lb // CB
    bb = lb % CB
    colF = bb * LSH + j

    F2 = np.zeros((NCORES, NCH, 2, 64, CB * LSH), np.float32)
    F2[core, ch, h, :, colF] = batch_feats[order]

    # points: fixed 32-fold split within each subbin
    f = r // LSP
    jp = r - f * LSP
    colP = lb * LSP + jp
    P32 = np.zeros((NCORES, 3, 32, NB * LSP), np.float32)
    P32[core, :, f, colP] = batch_points[order]

    # per-subbin inverse-count tables (replicated across partitions)
    invc_sb = np.zeros(NCORES * NB, np.float32)
    invc_sb[:TSB] = invc[sb2bin]
    invc_sb = invc_sb.reshape(NCORES, 1, NB)
    INVF = np.broadcast_to(invc_sb, (NCORES, 128, NB)).copy()
    INVP = np.broadcast_to(invc_sb, (NCORES, 96, NB)).copy()

    # --- device execution ---
    nc = _build_device_program(NB, LS, CB)
    in_maps = [
        {
            "f2": np.ascontiguousarray(F2[c].reshape(NCH, 128, CB * LSH)),
            "p32": np.ascontiguousarray(P32[c].reshape(96, NB * LSP)),
            "invf": np.ascontiguousarray(INVF[c]),
            "invp": np.ascontiguousarray(INVP[c]),
        }
        for c in range(NCORES)
    ]
    trace = os.environ.get("BGS_TRACE", "0") == "1"
    if trace:
        _install_ntff_hook()
    res = bass_utils.run_bass_kernel_spmd(
        nc, in_maps, core_ids=list(range(NCORES)), trace=trace
    )
    if trace:
        last_exec_time_ns = res.exec_time_ns

    # --- host finalize: combine halves/folds, then subbin partials -> bins ---
    OF = np.stack([res.results[c]["of"] for c in range(NCORES)])  # [8,128,NB]
    OP = np.stack([res.results[c]["op"] for c in range(NCORES)])  # [8,96,NB]
    feats_parts = OF.reshape(NCORES, 2, 64, NB)
    feats_sb = (
        feats_parts[:, 0].transpose(0, 2, 1) + feats_parts[:, 1].transpose(0, 2, 1)
    ).reshape(NCORES * NB, 64)  # per-subbin scaled sums
    pts_parts = OP.reshape(NCORES, 3, 32, NB)
    pts_sb = pts_parts.sum(axis=2).transpose(0, 2, 1).reshape(NCORES * NB, 3)

    pool_points = np.zeros((N, 3), np.float32)
    pool_feats = np.zeros((N, 64), np.float32)
    np.add.at(pool_feats, sb2bin, feats_sb[:TSB])
    np.add.at(pool_points, sb2bin, pts_sb[:TSB])

    key_u_padded = np.full(N, -1, dtype=key_u.dtype)
    key_u_padded[:U] = key_u
    pool_batch = _pool_batch(key_u_padded, B)

    return pool_points, pool_feats, pool_batch
